# revision 19
# baseline (speedup 1.0000x reference)
"""GCNConv Trainium2 kernel (fp8 DoubleRow spmm, aggregate-first).

Reference computation (all raw row-major reshapes):
    x_flat = x.reshape(-1, 64)                 # [960000, 64]
    h = (x_flat @ W).reshape(5000, 12288)
    agg = F @ h                                # [5000,5000] @ [5000,12288]
    out = agg.reshape(-1, 64) + bias           # [960000, 64]

Equivalently, with X2 = x.reshape(5000, 12288) and BD = blockdiag(W x192):
    out2 = F @ X2 @ BD + bias  =  (F @ X2) @ BD + bias

Sharding: the 12288-wide feature axis splits into 8 shards of 1536 columns,
one per NeuronCore; F and W are replicated, so there are no collectives.

The spmm dominates (614 of 622 GFLOP) and runs in fp8e4m3 with
perf_mode=DoubleRow (K=256 per instruction, 157 TF/s/core measured).
Aggregation runs FIRST (G = F' @ X2 with the raw input quantized to fp8),
then the small W transform runs as a second on-device stage on fp16 data;
this removes the transform->quantize->aggregate pipeline (and its PSUM
evacuation bottleneck) from the critical path entirely.

fp8's ~2% quantization noise is halved by mean-centering the filter on the
host: F = mu*ones + F', with F' in [-0.5, 0.5] quantized to fp8 and the
rank-1 term mu * colsum(h) computed exactly on the host and folded into the
per-column bias.

Per-core device kernel (outputs the transpose; host untransposes):
  phase A: G^T[c-tile, outv-chunk] = sum_k2 X2-tile^T @ F'^T-chunk
           stationary = X2 tiles [128, 2, 128] fp8 (host-pretransposed,
           zero-padded to 5120 vertices), moving = F'^T [128, 2, 500] fp8,
           DoubleRow chains of 20 contracting 256 vertices each into
           [128, 500] fp32 PSUM; chunks are 500 wide so the 5000 output
           vertices carry zero padding waste. PSUM evacuates to SBUF fp16.
  phase B: out^T tile = w2^T @ G^T tile per 128-column pair block
           (stationary w2 never changes); bias (incl. rank-1 term, varying
           along the partition dim) is fused into the PSUM->SBUF copy;
           fp32 [128, 500] tiles DMA to out^T [1536, 5000].
"""

import numpy as np

import concourse.bass as bass
import concourse.mybir as mybir
import concourse.tile as tile
from concourse import bacc
from concourse.bass_utils import run_bass_kernel_spmd

N_CORES = 8
NV = 5000            # vertex count
P = 128
NVP = 5120           # NV padded to a multiple of 256 (DoubleRow pairs)
KT = NVP // P        # 40 contraction tiles of 128
KT2 = KT // 2        # 20 DoubleRow contraction steps of 256
COLS_TOTAL = 12288   # B*T*c_out columns of the transformed feature matrix
COLS = COLS_TOTAL // N_CORES   # 1536 per core
CT = COLS // P       # 12 column-pair tiles of 128 per core
NCH = 10             # output-vertex chunks
CH = NV // NCH       # 500 output vertices per chunk (no padding)
CIN = 64
COUT = 64
FT0_SPLIT = 10       # chunk 0 of F'^T staged as 10 mini-tiles of 4 k-tiles
KT0 = KT // FT0_SPLIT

F16_DT = mybir.dt.float16
FP8_DT = mybir.dt.float8e4
FP8_NP = mybir.dt.np(mybir.dt.float8e4)   # ml_dtypes.float8_e4m3


def build_nc():
    nc = bacc.Bacc(None, target_bir_lowering=False)

    x2d = nc.dram_tensor("x2d", [CT, P, KT, P], FP8_DT, kind="ExternalInput")
    ftd = nc.dram_tensor("ftd", [P, NCH, KT, CH], FP8_DT, kind="ExternalInput")
    w2d = nc.dram_tensor("w2d", [P, P], F16_DT, kind="ExternalInput")
    biasd = nc.dram_tensor("biasd", [P, CT], mybir.dt.float32, kind="ExternalInput")
    outd = nc.dram_tensor("out", [COLS, NV], mybir.dt.float32, kind="ExternalOutput")

    with tile.TileContext(nc) as tc:
        with (
            tc.tile_pool(name="const", bufs=1) as const,
            tc.tile_pool(name="x2p", bufs=CT - 1) as x2p,
            tc.tile_pool(name="x2h", bufs=2) as x2h,
            tc.tile_pool(name="ft0p", bufs=FT0_SPLIT) as ft0p,
            tc.tile_pool(name="ftp", bufs=3) as ftp,
            tc.tile_pool(name="gp", bufs=6) as gpool,
            tc.tile_pool(name="obp", bufs=3) as obp,
        ):
            # Startup DMA order is the critical path: chain (ch=0, cc=0)
            # needs the first half of x2 piece 0 plus F'^T mini-tile s by
            # step k2=2s, chains cc>=1 need one x2 piece each ~4.3us.
            # Issues alternate between the two HWDGE engines (sync +
            # scalar) because each dma_start costs ~0.6us of issue time on
            # its engine, which would otherwise serialize the early
            # transfers behind ~9us of issue overhead.
            x2_tiles = {}
            ft0_tiles = []

            def stage_x2(cc, eng):
                t = x2p.tile([P, KT, P], FP8_DT, name="x2s")
                eng.dma_start(t[:], x2d[cc])
                x2_tiles[cc] = t

            def stage_ft0(s, eng):
                t = ft0p.tile([P, KT0, CH], FP8_DT, name="ft0")
                eng.dma_start(t[:], ftd[:, 0, s * KT0 : (s + 1) * KT0, :])
                ft0_tiles.append(t)

            # x2 piece 0 split in halves so chain 0 can start on the first;
            # transfers are issued in chain-0 consumption order (the
            # aggregate DMA bandwidth is the binding constraint for the
            # first two chains, so anything out of order directly stalls
            # the tensor engine).
            x2p0_lo = x2h.tile([P, KT // 2, P], FP8_DT, name="x2lo")
            nc.sync.dma_start(x2p0_lo[:], x2d[0, :, : KT // 2, :])
            stage_ft0(0, nc.sync)
            x2p0_hi = x2h.tile([P, KT // 2, P], FP8_DT, name="x2hi")
            nc.scalar.dma_start(x2p0_hi[:], x2d[0, :, KT // 2 :, :])
            stage_ft0(1, nc.sync)
            stage_ft0(2, nc.scalar)
            stage_ft0(3, nc.sync)
            stage_x2(1, nc.scalar)
            stage_ft0(4, nc.sync)
            stage_ft0(5, nc.scalar)
            stage_ft0(6, nc.sync)
            stage_x2(2, nc.scalar)
            stage_ft0(7, nc.sync)
            stage_ft0(8, nc.scalar)
            stage_ft0(9, nc.sync)

            w2_sb = const.tile([P, P], F16_DT)
            nc.scalar.dma_start(w2_sb[:], w2d[:])
            bias_sb = const.tile([P, CT], mybir.dt.float32)
            nc.sync.dma_start(bias_sb[:], biasd[:])

            for cc in range(3, CT):
                stage_x2(cc, nc.scalar if cc % 2 else nc.sync)

            ft_tiles = {}

            def stage_ft(ch):
                t = ftp.tile([P, KT, CH], FP8_DT, name="fts")
                nc.scalar.dma_start(t[:], ftd[:, ch])
                ft_tiles[ch] = t

            stage_ft(1)

            with (
                tc.tile_pool(name="psA", bufs=6, space="PSUM") as psA,
                tc.tile_pool(name="psB", bufs=2, space="PSUM") as psB,
            ):
                # B stage for tile (ch, cc): one matmul with the resident
                # w2 stationary, bias fused into the PSUM->SBUF evacuation.
                def emit_B(ch, cc, gt):
                    pb = psB.tile([P, CH], mybir.dt.float32, name="pb")
                    nc.tensor.matmul(
                        pb[:], w2_sb[:], gt[:], start=True, stop=True
                    )
                    ob = obp.tile([P, CH], mybir.dt.float32, name="ob")
                    if cc % 2 == 0:
                        nc.scalar.add(ob[:], pb[:], bias_sb[:, cc : cc + 1])
                    else:
                        nc.vector.tensor_scalar_add(
                            ob[:], pb[:], bias_sb[:, cc : cc + 1]
                        )
                    nc.sync.dma_start(
                        outd[cc * P : (cc + 1) * P, ch * CH : (ch + 1) * CH],
                        ob[:],
                    )

                # pending delays each B matmul by one chain (~4.3us) so its
                # wait on the G^T evacuation is already satisfied when it
                # reaches the head of the tensor queue (no HOL stall).
                pending = None
                for ch in range(NCH):
                    if 1 <= ch <= NCH - 2:
                        stage_ft(ch + 1)
                    for cc in range(CT):
                        ps = psA.tile([P, CH], mybir.dt.float32, name="ps")
                        for k2 in range(KT2):
                            if ch == 0:
                                t = ft0_tiles[(2 * k2) // KT0]
                                ft_sl = t[:, (2 * k2) % KT0 : (2 * k2) % KT0 + 2, :]
                            else:
                                ft_sl = ft_tiles[ch][:, 2 * k2 : 2 * k2 + 2, :]
                            if cc == 0:
                                if k2 < KT2 // 2:
                                    x_sl = x2p0_lo[:, 2 * k2 : 2 * k2 + 2, :]
                                else:
                                    o = 2 * k2 - KT // 2
                                    x_sl = x2p0_hi[:, o : o + 2, :]
                            else:
                                x_sl = x2_tiles[cc][:, 2 * k2 : 2 * k2 + 2, :]
                            nc.tensor.matmul(
                                ps[:],
                                x_sl,
                                ft_sl,
                                start=(k2 == 0),
                                stop=(k2 == KT2 - 1),
                                perf_mode=mybir.MatmulPerfMode.DoubleRow,
                            )
                        gt = gpool.tile([P, CH], F16_DT, name="gt")
                        # evac on the engine the bias-add for this cc does
                        # NOT use, so vector and scalar each carry one op
                        # per tile
                        if cc % 2 == 0:
                            nc.vector.tensor_copy(gt[:], ps[:])
                        else:
                            nc.scalar.copy(gt[:], ps[:])
                        if pending is not None:
                            emit_B(*pending)
                        pending = (ch, cc, gt)
                emit_B(*pending)

    nc.compile()
    return nc


def prepare_in_maps(x, gcnconv_filter, weight, bias):
    x2 = np.ascontiguousarray(x, dtype=np.float32).reshape(NV, COLS_TOTAL)

    f = np.asarray(gcnconv_filter, dtype=np.float32)
    mu = float(f.mean(dtype=np.float64))
    # F'^T padded to 5120 rows, quantized fp8, swizzled so that the staging
    # DMA for chunk ch reads [128, KT*CH] contiguously per partition:
    # ftd[p, ch, kt, j] = F'^T[kt*128 + p, ch*500 + j]
    ftp_ = np.zeros((NVP, NV), dtype=np.float32)
    ftp_[:NV, :] = (f - mu).T
    ft_sw = np.ascontiguousarray(
        ftp_.astype(FP8_NP).reshape(KT, P, NCH, CH).transpose(1, 2, 0, 3)
    )

    w2 = np.zeros((P, P), dtype=np.float16)
    w = np.asarray(weight, dtype=np.float32)
    w2[:CIN, :COUT] = w
    w2[CIN:, COUT:] = w

    # bias_tot[j] = bias[j % 64] + mu * colsum_h[j], with
    # colsum_h[block g] = (sum_v X2[v, g-block]) @ W  (exact, host fp64)
    colsum_x = x2.sum(axis=0, dtype=np.float64)                  # [12288]
    colsum_h = colsum_x.reshape(-1, CIN) @ w.astype(np.float64)  # [192, 64]
    bias_tot = (
        np.asarray(bias, dtype=np.float64)[None, :] + mu * colsum_h
    ).reshape(COLS_TOTAL).astype(np.float32)

    x2q = x2.astype(FP8_NP)
    in_maps = []
    for c in range(N_CORES):
        # x2d[cc, p, kt, m] = X2[kt*128 + p, core_off + cc*128 + m], padded
        xcq = np.zeros((NVP, COLS), dtype=FP8_NP)
        xcq[:NV] = x2q[:, c * COLS : (c + 1) * COLS]
        x2d = np.ascontiguousarray(
            xcq.reshape(KT, P, CT, P).transpose(2, 1, 0, 3)
        )
        biasb = np.ascontiguousarray(
            bias_tot[c * COLS : (c + 1) * COLS].reshape(CT, P).T
        )
        in_maps.append({"x2d": x2d, "ftd": ft_sw, "w2d": w2, "biasd": biasb})
    return in_maps


def assemble_output(results):
    out2 = np.empty((NV, COLS_TOTAL), dtype=np.float32)
    for c in range(N_CORES):
        out2[:, c * COLS : (c + 1) * COLS] = results[c]["out"].T
    return out2.reshape(NV * COLS_TOTAL // COUT, COUT)


_NC_CACHE = None


def kernel(x, gcnconv_filter, weight, bias):
    global _NC_CACHE
    if _NC_CACHE is None:
        _NC_CACHE = build_nc()
    in_maps = prepare_in_maps(x, gcnconv_filter, weight, bias)
    res = run_bass_kernel_spmd(_NC_CACHE, in_maps, core_ids=list(range(N_CORES)))
    return assemble_output(res.results)


# revision 21
# speedup vs baseline: 1.0046x; 1.0046x over previous
"""GCNConv Trainium2 kernel (fp8 DoubleRow spmm, aggregate-first).

Reference computation (all raw row-major reshapes):
    x_flat = x.reshape(-1, 64)                 # [960000, 64]
    h = (x_flat @ W).reshape(5000, 12288)
    agg = F @ h                                # [5000,5000] @ [5000,12288]
    out = agg.reshape(-1, 64) + bias           # [960000, 64]

Equivalently, with X2 = x.reshape(5000, 12288) and BD = blockdiag(W x192):
    out2 = F @ X2 @ BD + bias  =  (F @ X2) @ BD + bias

Sharding: the 12288-wide feature axis splits into 8 shards of 1536 columns,
one per NeuronCore; F and W are replicated, so there are no collectives.

The spmm dominates (614 of 622 GFLOP) and runs in fp8e4m3 with
perf_mode=DoubleRow (K=256 per instruction, 157 TF/s/core measured).
Aggregation runs FIRST (G = F' @ X2 with the raw input quantized to fp8),
then the small W transform runs as a second on-device stage on fp16 data;
this removes the transform->quantize->aggregate pipeline (and its PSUM
evacuation bottleneck) from the critical path entirely.

fp8's ~2% quantization noise is halved by mean-centering the filter on the
host: F = mu*ones + F', with F' in [-0.5, 0.5] quantized to fp8 and the
rank-1 term mu * colsum(h) computed exactly on the host and folded into the
per-column bias.

Per-core device kernel (outputs the transpose; host untransposes):
  phase A: G^T[c-tile, outv-chunk] = sum_k2 X2-tile^T @ F'^T-chunk
           stationary = X2 tiles [128, 2, 128] fp8 (host-pretransposed,
           zero-padded to 5120 vertices), moving = F'^T [128, 2, 500] fp8,
           DoubleRow chains of 20 contracting 256 vertices each into
           [128, 500] fp32 PSUM; chunks are 500 wide so the 5000 output
           vertices carry zero padding waste. PSUM evacuates to SBUF fp16.
  phase B: out^T tile = w2^T @ G^T tile per 128-column pair block
           (stationary w2 never changes); bias (incl. rank-1 term, varying
           along the partition dim) is fused into the PSUM->SBUF copy;
           fp32 [128, 500] tiles DMA to out^T [1536, 5000].
"""

import numpy as np

import concourse.bass as bass
import concourse.mybir as mybir
import concourse.tile as tile
from concourse import bacc
from concourse.bass_utils import run_bass_kernel_spmd

N_CORES = 8
NV = 5000            # vertex count
P = 128
NVP = 5120           # NV padded to a multiple of 256 (DoubleRow pairs)
KT = NVP // P        # 40 contraction tiles of 128
KT2 = KT // 2        # 20 DoubleRow contraction steps of 256
COLS_TOTAL = 12288   # B*T*c_out columns of the transformed feature matrix
COLS = COLS_TOTAL // N_CORES   # 1536 per core
CT = COLS // P       # 12 column-pair tiles of 128 per core
NCH = 10             # output-vertex chunks
CH = NV // NCH       # 500 output vertices per chunk (no padding)
CIN = 64
COUT = 64
FT0_SPLIT = 10       # chunk 0 of F'^T staged as 10 mini-tiles of 4 k-tiles
KT0 = KT // FT0_SPLIT

F16_DT = mybir.dt.float16
FP8_DT = mybir.dt.float8e4
FP8_NP = mybir.dt.np(mybir.dt.float8e4)   # ml_dtypes.float8_e4m3


def build_nc():
    nc = bacc.Bacc(None, target_bir_lowering=False)

    x2d = nc.dram_tensor("x2d", [CT, P, KT, P], FP8_DT, kind="ExternalInput")
    ftd = nc.dram_tensor("ftd", [P, NCH, KT, CH], FP8_DT, kind="ExternalInput")
    w2d = nc.dram_tensor("w2d", [P, P], F16_DT, kind="ExternalInput")
    biasd = nc.dram_tensor("biasd", [P, CT], mybir.dt.float32, kind="ExternalInput")
    outd = nc.dram_tensor("out", [COLS, NV], mybir.dt.float32, kind="ExternalOutput")

    with tile.TileContext(nc) as tc:
        with (
            tc.tile_pool(name="const", bufs=1) as const,
            tc.tile_pool(name="x2p", bufs=CT - 1) as x2p,
            tc.tile_pool(name="x2h", bufs=2) as x2h,
            tc.tile_pool(name="ft0p", bufs=FT0_SPLIT) as ft0p,
            tc.tile_pool(name="ftp", bufs=3) as ftp,
            tc.tile_pool(name="gp", bufs=6) as gpool,
            tc.tile_pool(name="obp", bufs=3) as obp,
        ):
            # Startup DMA order is the critical path: chain (ch=0, cc=0)
            # needs the first half of x2 piece 0 plus F'^T mini-tile s by
            # step k2=2s, chains cc>=1 need one x2 piece each ~4.3us.
            # Issues alternate between the two HWDGE engines (sync +
            # scalar) because each dma_start costs ~0.6us of issue time on
            # its engine, which would otherwise serialize the early
            # transfers behind ~9us of issue overhead.
            x2_tiles = {}
            ft0_tiles = []

            def stage_x2(cc, eng):
                t = x2p.tile([P, KT, P], FP8_DT, name="x2s")
                eng.dma_start(t[:], x2d[cc])
                x2_tiles[cc] = t

            def stage_ft0(s, eng):
                t = ft0p.tile([P, KT0, CH], FP8_DT, name="ft0")
                eng.dma_start(t[:], ftd[:, 0, s * KT0 : (s + 1) * KT0, :])
                ft0_tiles.append(t)

            # x2 piece 0 split in halves so chain 0 can start on the first;
            # transfers are issued in chain-0 consumption order (the
            # aggregate DMA bandwidth is the binding constraint for the
            # first two chains, so anything out of order directly stalls
            # the tensor engine).
            x2p0_lo = x2h.tile([P, KT // 2, P], FP8_DT, name="x2lo")
            nc.sync.dma_start(x2p0_lo[:], x2d[0, :, : KT // 2, :])
            stage_ft0(0, nc.sync)
            x2p0_hi = x2h.tile([P, KT // 2, P], FP8_DT, name="x2hi")
            nc.scalar.dma_start(x2p0_hi[:], x2d[0, :, KT // 2 :, :])
            stage_ft0(1, nc.sync)
            stage_ft0(2, nc.scalar)
            stage_ft0(3, nc.sync)
            stage_x2(1, nc.scalar)
            stage_ft0(4, nc.sync)
            stage_ft0(5, nc.scalar)
            stage_ft0(6, nc.sync)
            stage_x2(2, nc.scalar)
            stage_ft0(7, nc.sync)
            stage_ft0(8, nc.scalar)
            stage_ft0(9, nc.sync)

            w2_sb = const.tile([P, P], F16_DT)
            nc.scalar.dma_start(w2_sb[:], w2d[:])
            bias_sb = const.tile([P, CT], mybir.dt.float32)
            nc.sync.dma_start(bias_sb[:], biasd[:])

            for cc in range(3, CT):
                stage_x2(cc, nc.scalar if cc % 2 else nc.sync)

            ft_tiles = {}

            def stage_ft(ch):
                t = ftp.tile([P, KT, CH], FP8_DT, name="fts")
                nc.scalar.dma_start(t[:], ftd[:, ch])
                ft_tiles[ch] = t

            stage_ft(1)

            with (
                tc.tile_pool(name="psA", bufs=6, space="PSUM") as psA,
                tc.tile_pool(name="psB", bufs=2, space="PSUM") as psB,
            ):
                # B stage for tile (ch, cc): one matmul with the resident
                # w2 stationary, bias fused into the PSUM->SBUF evacuation.
                def emit_B(ch, cc, gt, off=0, width=CH):
                    pb = psB.tile([P, width], mybir.dt.float32, name="pb")
                    nc.tensor.matmul(
                        pb[:], w2_sb[:], gt[:], start=True, stop=True
                    )
                    ob = obp.tile([P, width], mybir.dt.float32, name="ob")
                    if cc % 2 == 0:
                        nc.scalar.add(ob[:], pb[:], bias_sb[:, cc : cc + 1])
                    else:
                        nc.vector.tensor_scalar_add(
                            ob[:], pb[:], bias_sb[:, cc : cc + 1]
                        )
                    nc.sync.dma_start(
                        outd[
                            cc * P : (cc + 1) * P,
                            ch * CH + off : ch * CH + off + width,
                        ],
                        ob[:],
                    )

                # pending delays each B matmul by one chain (~4.3us) so its
                # wait on the G^T evacuation is already satisfied when it
                # reaches the head of the tensor queue (no HOL stall).
                pending = None
                for ch in range(NCH):
                    if 1 <= ch <= NCH - 2:
                        stage_ft(ch + 1)
                    for cc in range(CT):
                        # The very last tile runs as two half-width chains
                        # so its evac/transform/bias/store pipeline hides
                        # under the second half instead of trailing the
                        # whole kernel serially.
                        halves = (
                            2 if (ch == NCH - 1 and cc == CT - 1) else 1
                        )
                        hw_ = CH // halves
                        for h in range(halves):
                            ps = psA.tile([P, hw_], mybir.dt.float32, name="ps")
                            for k2 in range(KT2):
                                if ch == 0:
                                    t = ft0_tiles[(2 * k2) // KT0]
                                    ft_sl = t[
                                        :,
                                        (2 * k2) % KT0 : (2 * k2) % KT0 + 2,
                                        h * hw_ : (h + 1) * hw_,
                                    ]
                                else:
                                    ft_sl = ft_tiles[ch][
                                        :,
                                        2 * k2 : 2 * k2 + 2,
                                        h * hw_ : (h + 1) * hw_,
                                    ]
                                if cc == 0:
                                    if k2 < KT2 // 2:
                                        x_sl = x2p0_lo[:, 2 * k2 : 2 * k2 + 2, :]
                                    else:
                                        o = 2 * k2 - KT // 2
                                        x_sl = x2p0_hi[:, o : o + 2, :]
                                else:
                                    x_sl = x2_tiles[cc][:, 2 * k2 : 2 * k2 + 2, :]
                                nc.tensor.matmul(
                                    ps[:],
                                    x_sl,
                                    ft_sl,
                                    start=(k2 == 0),
                                    stop=(k2 == KT2 - 1),
                                    perf_mode=mybir.MatmulPerfMode.DoubleRow,
                                )
                            gt = gpool.tile([P, hw_], F16_DT, name="gt")
                            # evac on the engine the bias-add for this cc
                            # does NOT use, so vector and scalar each carry
                            # one op per tile
                            if (cc + h) % 2 == 0:
                                nc.vector.tensor_copy(gt[:], ps[:])
                            else:
                                nc.scalar.copy(gt[:], ps[:])
                            if pending is not None:
                                emit_B(*pending)
                            pending = (ch, cc, gt, h * hw_, hw_)
                emit_B(*pending)

    nc.compile()
    return nc


def prepare_in_maps(x, gcnconv_filter, weight, bias):
    x2 = np.ascontiguousarray(x, dtype=np.float32).reshape(NV, COLS_TOTAL)

    f = np.asarray(gcnconv_filter, dtype=np.float32)
    mu = float(f.mean(dtype=np.float64))
    # F'^T padded to 5120 rows, quantized fp8, swizzled so that the staging
    # DMA for chunk ch reads [128, KT*CH] contiguously per partition:
    # ftd[p, ch, kt, j] = F'^T[kt*128 + p, ch*500 + j]
    ftp_ = np.zeros((NVP, NV), dtype=np.float32)
    ftp_[:NV, :] = (f - mu).T
    ft_sw = np.ascontiguousarray(
        ftp_.astype(FP8_NP).reshape(KT, P, NCH, CH).transpose(1, 2, 0, 3)
    )

    w2 = np.zeros((P, P), dtype=np.float16)
    w = np.asarray(weight, dtype=np.float32)
    w2[:CIN, :COUT] = w
    w2[CIN:, COUT:] = w

    # bias_tot[j] = bias[j % 64] + mu * colsum_h[j], with
    # colsum_h[block g] = (sum_v X2[v, g-block]) @ W  (exact, host fp64)
    colsum_x = x2.sum(axis=0, dtype=np.float64)                  # [12288]
    colsum_h = colsum_x.reshape(-1, CIN) @ w.astype(np.float64)  # [192, 64]
    bias_tot = (
        np.asarray(bias, dtype=np.float64)[None, :] + mu * colsum_h
    ).reshape(COLS_TOTAL).astype(np.float32)

    x2q = x2.astype(FP8_NP)
    in_maps = []
    for c in range(N_CORES):
        # x2d[cc, p, kt, m] = X2[kt*128 + p, core_off + cc*128 + m], padded
        xcq = np.zeros((NVP, COLS), dtype=FP8_NP)
        xcq[:NV] = x2q[:, c * COLS : (c + 1) * COLS]
        x2d = np.ascontiguousarray(
            xcq.reshape(KT, P, CT, P).transpose(2, 1, 0, 3)
        )
        biasb = np.ascontiguousarray(
            bias_tot[c * COLS : (c + 1) * COLS].reshape(CT, P).T
        )
        in_maps.append({"x2d": x2d, "ftd": ft_sw, "w2d": w2, "biasd": biasb})
    return in_maps


def assemble_output(results):
    out2 = np.empty((NV, COLS_TOTAL), dtype=np.float32)
    for c in range(N_CORES):
        out2[:, c * COLS : (c + 1) * COLS] = results[c]["out"].T
    return out2.reshape(NV * COLS_TOTAL // COUT, COUT)


_NC_CACHE = None


def kernel(x, gcnconv_filter, weight, bias):
    global _NC_CACHE
    if _NC_CACHE is None:
        _NC_CACHE = build_nc()
    in_maps = prepare_in_maps(x, gcnconv_filter, weight, bias)
    res = run_bass_kernel_spmd(_NC_CACHE, in_maps, core_ids=list(range(N_CORES)))
    return assemble_output(res.results)
